# revision 31
# baseline (speedup 1.0000x reference)
"""DetConB loss (nn_DetConBLoss) on 8 TRN2 NeuronCores via Bass/Tile.

Strategy (data-parallel over batch, targets replicated):
  - Host: l2-normalize preds/targets in f32, flatten to (4096, 256),
    transpose to (d, rows), cast fp8e4m3. Core c owns pred rows
    [c*512, (c+1)*512). Each core receives the full targets with columns
    rolled by c*512 so its own-image diagonal band sits at a fixed,
    compile-time-constant column range (the program is SPMD-identical).
    All inputs are packed into ONE dram tensor in the exact order the
    device consumes them, contiguous per partition, so the input DMA is
    a handful of large-descriptor segments striped over both HWDGE
    queues (a single queue moves only ~43 GB/s; small descriptors are
    far worse).
  - Device (per core): 52 units, each a (128 pred x W target) slab:
    fp8 DoubleRow matmuls (K=256 in one pass, f32 PSUM) + one of two
    row-sum consumers, statically balanced to the engines' measured
    rates (~51us each at the 1.2 GHz uncore state):
      * 30 ACT units (W=1536/512): exp via ScalarE ACTIVATE with the
        free in-op accumulator (accum_out) - one fused pass.
      * 22 DVE units (W=1024/512): Schraudolph fast-exp on DVE
        (int-converting multiply-add to an i32 whose bits are the f32
        exp) + bitcast tensor_reduce. Placed on target slabs that
        exclude both own-image diagonal bands, so the -inf correction
        on the host subtracts exact exps.
    Units are ordered target-range-major so the first 8 units of each
    engine reuse one input chunk. PSUM: ACT ping-pongs over its own
    2x[128,1536] slots; DVE needs only ONE [128,1024] slot (its refill
    hides under the previous unit's tensor_reduce, which reads SBUF).
    6+2 = 8 banks, and no slot is ever handed between engines.
  - Host: the 16x16 own-image diagonal dot blocks (recomputed from the
    same fp8 inputs, ~0.4% of total FLOPs), masks from the roi indices,
    positive-pair sums, the -inf masking correction (subtract the exp of
    masked entries from the denominators), log, and the final mean.
"""
import numpy as np
import ml_dtypes

import concourse.bacc as bacc
import concourse.mybir as mybir
import concourse.tile as tile
from concourse.bass_utils import run_bass_kernel_spmd

TEMP = 0.1
EPS = 1e-11
SCALE = float(np.float32(1.0 / (TEMP + EPS)))
NCORES = 8
B, N, D = 256, 16, 256
R = B * N          # 4096 flat rows
RPC = R // NCORES  # 512 rows per core
BF16 = mybir.dt.bfloat16
FP8 = mybir.dt.float8e4
NPFP8 = ml_dtypes.float8_e4m3
F32 = mybir.dt.float32
I32 = mybir.dt.int32
# Schraudolph fast-exp: exp(s*x) ~= bitcast_f32(int32(x*SA + SB))
SA = float(np.float32((2**23 / np.log(2.0)) * (1.0 / (0.1 + 1e-11))))
SB = float(np.float32(127 * 2**23 - 486411))

# Per (pt, tsel) side (4096 target cols), one of three layouts.  The all-D
# and hybrid sides must keep the own-image diagonal (aa: t1 cols [0,512)
# for px=0, bb: t2 for px=1) on an A slab - hybrids do (diag is in A:0-1536).
ALL_D_SIDES = {(3, 1), (7, 0)}
HYBRID_SIDES = {(1, 0)}


def side_units(pt, ts):
    if (pt, ts) in ALL_D_SIDES:
        return [(pt, ts, c, 1024, "D") for c in (0, 1024, 2048, 3072)]
    if (pt, ts) in HYBRID_SIDES:
        return [(pt, ts, 0, 1536, "A"), (pt, ts, 1536, 1536, "A"),
                (pt, ts, 3072, 512, "A"), (pt, ts, 3584, 512, "D")]
    return [(pt, ts, 0, 1536, "A"), (pt, ts, 1536, 1536, "A"),
            (pt, ts, 3072, 1024, "D")]


def ucost(w, kind):
    return (w + 352) / 1.2 + 190 if kind == "A" else 2.25 * w + 116


def unit_sequence():
    """Target-range-major queues per engine, merged by cumulative engine
    time so both engines are fed from the start and finish together."""
    units = [u for pt in range(8) for ts in range(2)
             for u in side_units(pt, ts)]
    a = sorted([u for u in units if u[4] == "A"],
               key=lambda u: (u[1], u[2], u[0]))
    d = sorted([u for u in units if u[4] == "D"],
               key=lambda u: (u[1], u[2], u[0]))
    seq, ta, td = [], 0.0, 0.0
    while a or d:
        ca = ta + ucost(a[0][3], "A") if a else None
        cd = td + ucost(d[0][3], "D") if d else None
        if a and (not d or ca <= cd):
            seq.append(a.pop(0))
            ta = ca
        else:
            seq.append(d.pop(0))
            td = cd
    return seq


def build_layout():
    """Pack order of the single input tensor: pred tiles (256B) at first
    use, 512-col target chunk-pairs (k0|k1, 1024B) at first use."""
    seq = unit_sequence()
    p_off, t_off, atoms = {}, {}, []
    off = 0
    for pt, ts, c0, w, _ in seq:
        if pt not in p_off:
            p_off[pt] = off
            atoms.append((off, 256))
            off += 256
        for c in range(c0, c0 + w, 512):
            if (ts, c) not in t_off:
                t_off[(ts, c)] = off
                atoms.append((off, 1024))
                off += 1024
    return seq, p_off, t_off, atoms, off


SEQ, P_OFF, T_OFF, ATOMS, INP_BYTES = build_layout()
UCOL = {u[:3]: i for i, u in enumerate(SEQ)}


def dma_segments():
    """Split the packed stream into segments cut exactly at the first two
    units' data boundaries (seg0 = all of unit 0's data, first on the SP
    queue; seg1 = unit 1's, first on the ACT queue - so the scheduler's
    DMA-readiness model orders the first consumers first), then
    ~4KB-per-partition runs striped over both queues."""
    # seg0 = unit 0's data (SP queue), seg1 = just unit 1's 256B pred tile
    # (ACT queue - segment-completion semaphores are all-or-nothing, so a
    # small segment un-gates the first DVE unit ~1.5us earlier), seg2 = the
    # rest of unit 2's data.
    early = [3328, 3584, 5376]
    cuts, acc = [], 0
    for off, size in ATOMS:
        acc += size
        if len(cuts) < len(early) and acc >= early[len(cuts)]:
            cuts.append(off + size)
        elif len(cuts) >= len(early) and acc - cuts[-1] >= 4096:
            cuts.append(off + size)
    if not cuts or cuts[-1] != INP_BYTES:
        cuts.append(INP_BYTES)
    segs, lo = [], 0
    for hi in cuts:
        segs.append((lo, hi))
        lo = hi
    return segs


def build_nc():
    """Build + schedule + compile the SPMD per-core Bass program."""
    nc = bacc.Bacc("TRN2", target_bir_lowering=False, debug=False,
                   num_devices=NCORES)

    inp_dram = nc.dram_tensor("inp", [128, INP_BYTES], FP8,
                              kind="ExternalInput")
    sacc = nc.dram_tensor("sacc", [128, 64], F32, kind="ExternalOutput")

    with tile.TileContext(nc) as tc:
        with (
            tc.tile_pool(name="const", bufs=1) as const_pool,
            tc.tile_pool(name="psum", bufs=1, space="PSUM") as psum_pool,
        ):
            inp_sb = const_pool.tile([128, INP_BYTES], FP8, name="inp_sb",
                                     tag="inp")
            strip = const_pool.tile([128, 64], F32, name="strip", tag="strip")
            zbias = const_pool.tile([128, 1], F32, name="zbias", tag="zbias")
            scr = const_pool.tile([128, 1536], BF16, name="scr", tag="scr")
            # manual 2-buffer rotation (a dedicated pool would add another
            # pool-teardown barrier to the exit path)
            schs = [const_pool.tile([128, 1024], I32, name=f"sch{i}",
                                    tag=f"sch{i}") for i in range(2)]
            # Input DMA segments first in program order, striped over the two
            # HWDGE queues (the early ACT-queue dma_start issues run before
            # the warm-up ACTIVATE and overlap the data wait).
            for i, (lo, hi) in enumerate(dma_segments()):
                eng = nc.scalar if i in (1, 3, 5) else nc.sync
                eng.dma_start(out=inp_sb[:, lo:hi], in_=inp_dram[:, lo:hi])

            # Explicit zero-bias AP, zeroed ON ScalarE so the dependency is
            # engine-internal program order (a DVE memset would chain the
            # first ACTIVATE behind a coarsened cross-engine semaphore that
            # only clears at the DVE's second Schraudolph op, ~3.5us late).
            # The strip needs NO init at all: accum_out and tensor_reduce
            # overwrite their outputs.
            nc.scalar.memzero(zbias)

            # PSUM: ACT pair 2x[128,1536] (6 banks) + one DVE slot (2 banks).
            psA = [psum_pool.tile([128, 1536], F32, name=f"psA{i}", tag=f"psA{i}")
                   for i in range(2)]
            psD = psum_pool.tile([128, 1024], F32, name="psD", tag="psD")

            # Warm the exp table set during the input-DMA window so the first
            # real ACTIVATE does not pay the ~2.7us ACT_TABLE_LOAD (input and
            # output are scratch - exp of garbage, never read).
            warm = const_pool.tile([128, 2], F32, name="warm", tag="warm")
            nc.scalar.memzero(warm)
            nc.scalar.activation(warm, warm,
                                 mybir.ActivationFunctionType.Exp, bias=zbias)
            # Zero the strip's never-written spare columns on ScalarE too
            # (same-engine ordering, no semaphore on the first EXP's path).
            nc.scalar.memzero(strip)

            def lhs_ap(pt):
                o = P_OFF[pt]
                return inp_sb[:, o:o + 256].rearrange("p (k c) -> p k c", k=2)

            def rhs_ap(ts, c):
                # 512-col moving chunk-pair (k0|k1) at (ts, c)
                o = T_OFF[(ts, c)]
                return inp_sb[:, o:o + 1024].rearrange("p (k c) -> p k c", k=2)

            na = nd = 0
            for i, (pt, ts, c0, w, kind) in enumerate(SEQ):
                ps = psA[na % 2] if kind == "A" else psD
                lhs = lhs_ap(pt)
                # fp8 DoubleRow: both 128-deep K chunks contract in a single
                # pass (lhsT/rhs carry the k pair on a middle AP dim).
                for j in range(0, w, 512):
                    nc.tensor.matmul(
                        ps[:, j:j + 512],
                        lhs, rhs_ap(ts, c0 + j),
                        start=True, stop=True,
                        perf_mode=mybir.MatmulPerfMode.DoubleRow)
                if kind == "A":
                    nc.scalar.activation(
                        scr[:, 0:w], ps[:, 0:w],
                        mybir.ActivationFunctionType.Exp,
                        bias=zbias, scale=SCALE,
                        accum_out=strip[:, i:i + 1])
                    na += 1
                else:
                    sch = schs[nd % 2]
                    nd += 1
                    nc.vector.tensor_scalar(
                        sch[:, 0:w], ps[:, 0:w], SA, SB,
                        op0=mybir.AluOpType.mult, op1=mybir.AluOpType.add)
                    nc.vector.tensor_reduce(
                        strip[:, i:i + 1], sch.bitcast(F32)[:, 0:w],
                        axis=mybir.AxisListType.X, op=mybir.AluOpType.add)
            # Final strip DMA on the sync HWDGE queue: drains in ~0.1us at
            # kernel exit (the gpsimd SWDGE path would take ~2.4us).
            nc.sync.dma_start(out=sacc.ap(), in_=strip)

    nc.compile()
    return nc


_NC = None


def _get_nc():
    global _NC
    if _NC is None:
        _NC = build_nc()
    return _NC


def _l2norm(x):
    return x / np.linalg.norm(x, axis=-1, keepdims=True)


def prep_arrays(pred1, pred2, target1, target2):
    """fp8 transposed operands, shared by host_prep and the sim check."""
    p1t = _l2norm(np.asarray(pred1, np.float32)).reshape(R, D).T.astype(NPFP8)
    p2t = _l2norm(np.asarray(pred2, np.float32)).reshape(R, D).T.astype(NPFP8)
    t1t = _l2norm(np.asarray(target1, np.float32)).reshape(R, D).T.astype(NPFP8)
    t2t = _l2norm(np.asarray(target2, np.float32)).reshape(R, D).T.astype(NPFP8)
    return p1t, p2t, t1t, t2t


def pack_core(p1t, p2t, t1t, t2t, c):
    """Pack core c's inputs into the need-ordered [128, INP_BYTES] tensor."""
    r0 = c * RPC
    t = [np.concatenate([t1t[:, r0:], t1t[:, :r0]], axis=1),
         np.concatenate([t2t[:, r0:], t2t[:, :r0]], axis=1)]
    pcat = np.concatenate([p1t[:, r0:r0 + RPC], p2t[:, r0:r0 + RPC]], axis=1)
    inp = np.zeros((128, INP_BYTES), NPFP8)
    for pt, o in P_OFF.items():
        for k in range(2):
            inp[:, o + k * 128:o + (k + 1) * 128] = \
                pcat[k * 128:(k + 1) * 128, pt * 128:(pt + 1) * 128]
    for (ts, cc), o in T_OFF.items():
        for k in range(2):
            inp[:, o + k * 512:o + (k + 1) * 512] = \
                t[ts][k * 128:(k + 1) * 128, cc:cc + 512]
    return inp


def host_prep(pred1, pred2, target1, target2):
    p1t, p2t, t1t, t2t = prep_arrays(pred1, pred2, target1, target2)
    # Raw own-image diagonal dot blocks (b, n, m), fp8-quantized operands in
    # f32 - the same products the device computes, ~0.4% of total FLOPs.
    pf = [p1t.T.astype(np.float32).reshape(B, N, D),
          p2t.T.astype(np.float32).reshape(B, N, D)]
    tf = [t1t.T.astype(np.float32).reshape(B, N, D),
          t2t.T.astype(np.float32).reshape(B, N, D)]
    diag = [[np.einsum('bnd,bmd->bnm', pf[px], tf[ts]).astype(np.float32)
             for ts in range(2)] for px in range(2)]
    in_maps = [{"inp": pack_core(p1t, p2t, t1t, t2t, c)}
               for c in range(NCORES)]
    return in_maps, diag


def host_post(results, diag, pind1, pind2, tind1, tind2):
    # S[px, pred]: denominator sums of exp over all 8192 targets.
    S = np.zeros((2, R), np.float64)
    cols = {pt: [c for (p_, t_, c0), c in UCOL.items() if p_ == pt]
            for pt in range(8)}
    for c, res in enumerate(results):
        r0 = c * RPC
        sa = np.asarray(res["sacc"]).astype(np.float64)   # [128, 64]
        for pt in range(8):
            px, mt = pt // 4, pt % 4
            rows = r0 + mt * 128
            S[px, rows:rows + 128] += sa[:, cols[pt]].sum(axis=1)

    sc = np.float32(SCALE)
    D_aa = sc * diag[0][0]
    D_ab = sc * diag[0][1]
    D_ba = sc * diag[1][0]
    D_bb = sc * diag[1][1]

    f32 = np.float32
    pind1, pind2 = np.asarray(pind1), np.asarray(pind2)
    tind1, tind2 = np.asarray(tind1), np.asarray(tind2)
    same_aa = (pind1[:, :, None] == tind1[:, None, :]).astype(f32)
    same_ab = (pind1[:, :, None] == tind2[:, None, :]).astype(f32)
    same_ba = (pind2[:, :, None] == tind1[:, None, :]).astype(f32)
    same_bb = (pind2[:, :, None] == tind2[:, None, :]).astype(f32)

    S0 = S[0].reshape(B, N)
    S1 = S[1].reshape(B, N)
    # -inf masking correction: both diagonal bands live on ACT units, so
    # the device added exact exps - subtract exact exps.
    corr0 = (same_aa * np.exp(D_aa.astype(np.float64))).sum(-1)
    corr1 = (same_bb * np.exp(D_bb.astype(np.float64))).sum(-1)
    lse0 = np.log(S0 - corr0)
    lse1 = np.log(S1 - corr1)

    num_pos0 = same_ab.sum(-1)
    num_pos1 = same_ba.sum(-1)
    pos_sum0 = (same_ab * D_ab).sum(-1)
    pos_sum1 = (same_ba * D_ba).sum(-1)

    area0 = (pind1[:, :, None] == pind1[:, None, :]).astype(f32).sum(-1)
    area1 = (pind2[:, :, None] == pind2[:, None, :]).astype(f32).sum(-1)
    w0 = (num_pos0 > 0.001).astype(f32) / area0
    w1 = (num_pos1 > 0.001).astype(f32) / area1

    ce0 = -w0 * (pos_sum0 - num_pos0 * lse0) / np.maximum(num_pos0, 1.0)
    ce1 = -w1 * (pos_sum1 - num_pos1 * lse1) / np.maximum(num_pos1, 1.0)
    return np.float32(ce0.mean() + ce1.mean())


def run_hw(inputs, trace=False):
    nc = _get_nc()
    in_maps, diag = host_prep(inputs["pred1"], inputs["pred2"],
                              inputs["target1"], inputs["target2"])
    last_err = None
    for attempt in range(3):
        try:
            res = run_bass_kernel_spmd(nc, in_maps,
                                       core_ids=list(range(NCORES)),
                                       trace=trace)
            break
        except Exception as e:  # transient NRT device errors recover on retry
            last_err = e
            import time
            time.sleep(20 * (attempt + 1))
    else:
        raise last_err
    loss = host_post(res.results, diag, inputs["pind1"],
                     inputs["pind2"], inputs["tind1"], inputs["tind2"])
    return loss, res


def kernel(**inputs):
    loss, _ = run_hw(inputs, trace=False)
    return loss


# revision 32
# speedup vs baseline: 1.0002x; 1.0002x over previous
"""DetConB loss (nn_DetConBLoss) on 8 TRN2 NeuronCores via Bass/Tile.

Strategy (data-parallel over batch, targets replicated):
  - Host: l2-normalize preds/targets in f32, flatten to (4096, 256),
    transpose to (d, rows), cast fp8e4m3. Core c owns pred rows
    [c*512, (c+1)*512). Each core receives the full targets with columns
    rolled by c*512 so its own-image diagonal band sits at a fixed,
    compile-time-constant column range (the program is SPMD-identical).
    All inputs are packed into ONE dram tensor in the exact order the
    device consumes them, contiguous per partition, so the input DMA is
    a handful of large-descriptor segments striped over both HWDGE
    queues (a single queue moves only ~43 GB/s; small descriptors are
    far worse).
  - Device (per core): 52 units, each a (128 pred x W target) slab:
    fp8 DoubleRow matmuls (K=256 in one pass, f32 PSUM) + one of two
    row-sum consumers, statically balanced to the engines' measured
    rates (~51us each at the 1.2 GHz uncore state):
      * 30 ACT units (W=1536/512): exp via ScalarE ACTIVATE with the
        free in-op accumulator (accum_out) - one fused pass.
      * 22 DVE units (W=1024/512): Schraudolph fast-exp on DVE
        (int-converting multiply-add to an i32 whose bits are the f32
        exp) + bitcast tensor_reduce. Placed on target slabs that
        exclude both own-image diagonal bands, so the -inf correction
        on the host subtracts exact exps.
    Units are ordered target-range-major so the first 8 units of each
    engine reuse one input chunk. PSUM: ACT ping-pongs over its own
    2x[128,1536] slots; DVE needs only ONE [128,1024] slot (its refill
    hides under the previous unit's tensor_reduce, which reads SBUF).
    6+2 = 8 banks, and no slot is ever handed between engines.
  - Host: the 16x16 own-image diagonal dot blocks (recomputed from the
    same fp8 inputs, ~0.4% of total FLOPs), masks from the roi indices,
    positive-pair sums, the -inf masking correction (subtract the exp of
    masked entries from the denominators), log, and the final mean.
"""
import numpy as np
import ml_dtypes

import concourse.bacc as bacc
import concourse.mybir as mybir
import concourse.tile as tile
from concourse.bass_utils import run_bass_kernel_spmd

TEMP = 0.1
EPS = 1e-11
SCALE = float(np.float32(1.0 / (TEMP + EPS)))
NCORES = 8
B, N, D = 256, 16, 256
R = B * N          # 4096 flat rows
RPC = R // NCORES  # 512 rows per core
BF16 = mybir.dt.bfloat16
FP8 = mybir.dt.float8e4
NPFP8 = ml_dtypes.float8_e4m3
F32 = mybir.dt.float32
I32 = mybir.dt.int32
# Schraudolph fast-exp: exp(s*x) ~= bitcast_f32(int32(x*SA + SB))
SA = float(np.float32((2**23 / np.log(2.0)) * (1.0 / (0.1 + 1e-11))))
SB = float(np.float32(127 * 2**23 - 486411))

# Per (pt, tsel) side (4096 target cols), one of three layouts.  The all-D
# and hybrid sides must keep the own-image diagonal (aa: t1 cols [0,512)
# for px=0, bb: t2 for px=1) on an A slab - hybrids do (diag is in A:0-1536).
ALL_D_SIDES = {(3, 1), (7, 0)}
HYBRID_SIDES = {(1, 0)}


def side_units(pt, ts):
    if (pt, ts) in ALL_D_SIDES:
        return [(pt, ts, c, 1024, "D") for c in (0, 1024, 2048, 3072)]
    if (pt, ts) in HYBRID_SIDES:
        return [(pt, ts, 0, 1536, "A"), (pt, ts, 1536, 1536, "A"),
                (pt, ts, 3072, 512, "A"), (pt, ts, 3584, 512, "D")]
    return [(pt, ts, 0, 1536, "A"), (pt, ts, 1536, 1536, "A"),
            (pt, ts, 3072, 1024, "D")]


def ucost(w, kind):
    return (w + 352) / 1.2 + 190 if kind == "A" else 2.25 * w + 116


def unit_sequence():
    """Target-range-major queues per engine, merged by cumulative engine
    time so both engines are fed from the start and finish together."""
    units = [u for pt in range(8) for ts in range(2)
             for u in side_units(pt, ts)]
    a = sorted([u for u in units if u[4] == "A"],
               key=lambda u: (u[1], u[2], u[0]))
    d = sorted([u for u in units if u[4] == "D"],
               key=lambda u: (u[1], u[2], u[0]))
    seq, ta, td = [], 0.0, 0.0
    while a or d:
        ca = ta + ucost(a[0][3], "A") if a else None
        cd = td + ucost(d[0][3], "D") if d else None
        if a and (not d or ca <= cd):
            seq.append(a.pop(0))
            ta = ca
        else:
            seq.append(d.pop(0))
            td = cd
    return seq


def build_layout():
    """Pack order of the single input tensor: pred tiles (256B) at first
    use, 512-col target chunk-pairs (k0|k1, 1024B) at first use."""
    seq = unit_sequence()
    p_off, t_off, atoms = {}, {}, []
    off = 0
    for pt, ts, c0, w, _ in seq:
        if pt not in p_off:
            p_off[pt] = off
            atoms.append((off, 256))
            off += 256
        for c in range(c0, c0 + w, 512):
            if (ts, c) not in t_off:
                t_off[(ts, c)] = off
                atoms.append((off, 1024))
                off += 1024
    return seq, p_off, t_off, atoms, off


SEQ, P_OFF, T_OFF, ATOMS, INP_BYTES = build_layout()
UCOL = {u[:3]: i for i, u in enumerate(SEQ)}


def dma_segments():
    """Split the packed stream into segments cut exactly at the first two
    units' data boundaries (seg0 = all of unit 0's data, first on the SP
    queue; seg1 = unit 1's, first on the ACT queue - so the scheduler's
    DMA-readiness model orders the first consumers first), then
    ~4KB-per-partition runs striped over both queues."""
    early = [3328, 5376]
    cuts, acc = [], 0
    for off, size in ATOMS:
        acc += size
        if len(cuts) < len(early) and acc >= early[len(cuts)]:
            cuts.append(off + size)
        elif len(cuts) >= len(early) and acc - cuts[-1] >= 4096:
            cuts.append(off + size)
    if not cuts or cuts[-1] != INP_BYTES:
        cuts.append(INP_BYTES)
    segs, lo = [], 0
    for hi in cuts:
        segs.append((lo, hi))
        lo = hi
    return segs


def build_nc():
    """Build + schedule + compile the SPMD per-core Bass program."""
    nc = bacc.Bacc("TRN2", target_bir_lowering=False, debug=False,
                   num_devices=NCORES)

    inp_dram = nc.dram_tensor("inp", [128, INP_BYTES], FP8,
                              kind="ExternalInput")
    sacc = nc.dram_tensor("sacc", [128, 64], F32, kind="ExternalOutput")

    with tile.TileContext(nc) as tc:
        with (
            tc.tile_pool(name="const", bufs=1) as const_pool,
            tc.tile_pool(name="psum", bufs=1, space="PSUM") as psum_pool,
        ):
            inp_sb = const_pool.tile([128, INP_BYTES], FP8, name="inp_sb",
                                     tag="inp")
            strip = const_pool.tile([128, 64], F32, name="strip", tag="strip")
            zbias = const_pool.tile([128, 1], F32, name="zbias", tag="zbias")
            scr = const_pool.tile([128, 1536], BF16, name="scr", tag="scr")
            # manual 2-buffer rotation (a dedicated pool would add another
            # pool-teardown barrier to the exit path)
            schs = [const_pool.tile([128, 1024], I32, name=f"sch{i}",
                                    tag=f"sch{i}") for i in range(2)]
            # Input DMA segments first in program order, striped over the two
            # HWDGE queues (the early ACT-queue dma_start issues run before
            # the warm-up ACTIVATE and overlap the data wait).
            for i, (lo, hi) in enumerate(dma_segments()):
                eng = nc.scalar if i in (1, 3, 5) else nc.sync
                eng.dma_start(out=inp_sb[:, lo:hi], in_=inp_dram[:, lo:hi])

            # Explicit zero-bias AP, zeroed ON ScalarE so the dependency is
            # engine-internal program order (a DVE memset would chain the
            # first ACTIVATE behind a coarsened cross-engine semaphore that
            # only clears at the DVE's second Schraudolph op, ~3.5us late).
            # The strip needs NO init at all: accum_out and tensor_reduce
            # overwrite their outputs.
            nc.scalar.memzero(zbias)

            # PSUM: ACT pair 2x[128,1536] (6 banks) + one DVE slot (2 banks).
            psA = [psum_pool.tile([128, 1536], F32, name=f"psA{i}", tag=f"psA{i}")
                   for i in range(2)]
            psD = psum_pool.tile([128, 1024], F32, name="psD", tag="psD")

            # Warm the exp table set during the input-DMA window so the first
            # real ACTIVATE does not pay the ~2.7us ACT_TABLE_LOAD (input and
            # output are scratch - exp of garbage, never read).
            warm = const_pool.tile([128, 2], F32, name="warm", tag="warm")
            nc.scalar.memzero(warm)
            nc.scalar.activation(warm, warm,
                                 mybir.ActivationFunctionType.Exp, bias=zbias)
            # Zero the strip's never-written spare columns on ScalarE too
            # (same-engine ordering, no semaphore on the first EXP's path).
            nc.scalar.memzero(strip)

            def lhs_ap(pt):
                o = P_OFF[pt]
                return inp_sb[:, o:o + 256].rearrange("p (k c) -> p k c", k=2)

            def rhs_ap(ts, c):
                # 512-col moving chunk-pair (k0|k1) at (ts, c)
                o = T_OFF[(ts, c)]
                return inp_sb[:, o:o + 1024].rearrange("p (k c) -> p k c", k=2)

            na = nd = 0
            for i, (pt, ts, c0, w, kind) in enumerate(SEQ):
                ps = psA[na % 2] if kind == "A" else psD
                lhs = lhs_ap(pt)
                # fp8 DoubleRow: both 128-deep K chunks contract in a single
                # pass (lhsT/rhs carry the k pair on a middle AP dim).
                for j in range(0, w, 512):
                    nc.tensor.matmul(
                        ps[:, j:j + 512],
                        lhs, rhs_ap(ts, c0 + j),
                        start=True, stop=True,
                        perf_mode=mybir.MatmulPerfMode.DoubleRow)
                if kind == "A":
                    nc.scalar.activation(
                        scr[:, 0:w], ps[:, 0:w],
                        mybir.ActivationFunctionType.Exp,
                        bias=zbias, scale=SCALE,
                        accum_out=strip[:, i:i + 1])
                    na += 1
                else:
                    sch = schs[nd % 2]
                    nd += 1
                    nc.vector.tensor_scalar(
                        sch[:, 0:w], ps[:, 0:w], SA, SB,
                        op0=mybir.AluOpType.mult, op1=mybir.AluOpType.add)
                    nc.vector.tensor_reduce(
                        strip[:, i:i + 1], sch.bitcast(F32)[:, 0:w],
                        axis=mybir.AxisListType.X, op=mybir.AluOpType.add)
            # Final strip DMA on the sync HWDGE queue: drains in ~0.1us at
            # kernel exit (the gpsimd SWDGE path would take ~2.4us).
            nc.sync.dma_start(out=sacc.ap(), in_=strip)

    nc.compile()
    return nc


_NC = None


def _get_nc():
    global _NC
    if _NC is None:
        _NC = build_nc()
    return _NC


def _l2norm(x):
    return x / np.linalg.norm(x, axis=-1, keepdims=True)


def prep_arrays(pred1, pred2, target1, target2):
    """fp8 transposed operands, shared by host_prep and the sim check."""
    p1t = _l2norm(np.asarray(pred1, np.float32)).reshape(R, D).T.astype(NPFP8)
    p2t = _l2norm(np.asarray(pred2, np.float32)).reshape(R, D).T.astype(NPFP8)
    t1t = _l2norm(np.asarray(target1, np.float32)).reshape(R, D).T.astype(NPFP8)
    t2t = _l2norm(np.asarray(target2, np.float32)).reshape(R, D).T.astype(NPFP8)
    return p1t, p2t, t1t, t2t


def pack_core(p1t, p2t, t1t, t2t, c):
    """Pack core c's inputs into the need-ordered [128, INP_BYTES] tensor."""
    r0 = c * RPC
    t = [np.concatenate([t1t[:, r0:], t1t[:, :r0]], axis=1),
         np.concatenate([t2t[:, r0:], t2t[:, :r0]], axis=1)]
    pcat = np.concatenate([p1t[:, r0:r0 + RPC], p2t[:, r0:r0 + RPC]], axis=1)
    inp = np.zeros((128, INP_BYTES), NPFP8)
    for pt, o in P_OFF.items():
        for k in range(2):
            inp[:, o + k * 128:o + (k + 1) * 128] = \
                pcat[k * 128:(k + 1) * 128, pt * 128:(pt + 1) * 128]
    for (ts, cc), o in T_OFF.items():
        for k in range(2):
            inp[:, o + k * 512:o + (k + 1) * 512] = \
                t[ts][k * 128:(k + 1) * 128, cc:cc + 512]
    return inp


def host_prep(pred1, pred2, target1, target2):
    p1t, p2t, t1t, t2t = prep_arrays(pred1, pred2, target1, target2)
    # Raw own-image diagonal dot blocks (b, n, m), fp8-quantized operands in
    # f32 - the same products the device computes, ~0.4% of total FLOPs.
    pf = [p1t.T.astype(np.float32).reshape(B, N, D),
          p2t.T.astype(np.float32).reshape(B, N, D)]
    tf = [t1t.T.astype(np.float32).reshape(B, N, D),
          t2t.T.astype(np.float32).reshape(B, N, D)]
    diag = [[np.einsum('bnd,bmd->bnm', pf[px], tf[ts]).astype(np.float32)
             for ts in range(2)] for px in range(2)]
    in_maps = [{"inp": pack_core(p1t, p2t, t1t, t2t, c)}
               for c in range(NCORES)]
    return in_maps, diag


def host_post(results, diag, pind1, pind2, tind1, tind2):
    # S[px, pred]: denominator sums of exp over all 8192 targets.
    S = np.zeros((2, R), np.float64)
    cols = {pt: [c for (p_, t_, c0), c in UCOL.items() if p_ == pt]
            for pt in range(8)}
    for c, res in enumerate(results):
        r0 = c * RPC
        sa = np.asarray(res["sacc"]).astype(np.float64)   # [128, 64]
        for pt in range(8):
            px, mt = pt // 4, pt % 4
            rows = r0 + mt * 128
            S[px, rows:rows + 128] += sa[:, cols[pt]].sum(axis=1)

    sc = np.float32(SCALE)
    D_aa = sc * diag[0][0]
    D_ab = sc * diag[0][1]
    D_ba = sc * diag[1][0]
    D_bb = sc * diag[1][1]

    f32 = np.float32
    pind1, pind2 = np.asarray(pind1), np.asarray(pind2)
    tind1, tind2 = np.asarray(tind1), np.asarray(tind2)
    same_aa = (pind1[:, :, None] == tind1[:, None, :]).astype(f32)
    same_ab = (pind1[:, :, None] == tind2[:, None, :]).astype(f32)
    same_ba = (pind2[:, :, None] == tind1[:, None, :]).astype(f32)
    same_bb = (pind2[:, :, None] == tind2[:, None, :]).astype(f32)

    S0 = S[0].reshape(B, N)
    S1 = S[1].reshape(B, N)
    # -inf masking correction: both diagonal bands live on ACT units, so
    # the device added exact exps - subtract exact exps.
    corr0 = (same_aa * np.exp(D_aa.astype(np.float64))).sum(-1)
    corr1 = (same_bb * np.exp(D_bb.astype(np.float64))).sum(-1)
    lse0 = np.log(S0 - corr0)
    lse1 = np.log(S1 - corr1)

    num_pos0 = same_ab.sum(-1)
    num_pos1 = same_ba.sum(-1)
    pos_sum0 = (same_ab * D_ab).sum(-1)
    pos_sum1 = (same_ba * D_ba).sum(-1)

    area0 = (pind1[:, :, None] == pind1[:, None, :]).astype(f32).sum(-1)
    area1 = (pind2[:, :, None] == pind2[:, None, :]).astype(f32).sum(-1)
    w0 = (num_pos0 > 0.001).astype(f32) / area0
    w1 = (num_pos1 > 0.001).astype(f32) / area1

    ce0 = -w0 * (pos_sum0 - num_pos0 * lse0) / np.maximum(num_pos0, 1.0)
    ce1 = -w1 * (pos_sum1 - num_pos1 * lse1) / np.maximum(num_pos1, 1.0)
    return np.float32(ce0.mean() + ce1.mean())


def run_hw(inputs, trace=False):
    nc = _get_nc()
    in_maps, diag = host_prep(inputs["pred1"], inputs["pred2"],
                              inputs["target1"], inputs["target2"])
    last_err = None
    for attempt in range(3):
        try:
            res = run_bass_kernel_spmd(nc, in_maps,
                                       core_ids=list(range(NCORES)),
                                       trace=trace)
            break
        except Exception as e:  # transient NRT device errors recover on retry
            last_err = e
            import time
            time.sleep(20 * (attempt + 1))
    else:
        raise last_err
    loss = host_post(res.results, diag, inputs["pind1"],
                     inputs["pind2"], inputs["tind1"], inputs["tind2"])
    return loss, res


def kernel(**inputs):
    loss, _ = run_hw(inputs, trace=False)
    return loss


# revision 34
# speedup vs baseline: 1.0075x; 1.0073x over previous
"""DetConB loss (nn_DetConBLoss) on 8 TRN2 NeuronCores via Bass/Tile.

Strategy (data-parallel over batch, targets replicated):
  - Host: l2-normalize preds/targets in f32, flatten to (4096, 256),
    transpose to (d, rows), cast fp8e4m3. Core c owns pred rows
    [c*512, (c+1)*512). Each core receives the full targets with columns
    rolled by c*512 so its own-image diagonal band sits at a fixed,
    compile-time-constant column range (the program is SPMD-identical).
    All inputs are packed into ONE dram tensor in the exact order the
    device consumes them, contiguous per partition, so the input DMA is
    a handful of large-descriptor segments striped over both HWDGE
    queues (a single queue moves only ~43 GB/s; small descriptors are
    far worse).
  - Device (per core): 52 units, each a (128 pred x W target) slab:
    fp8 DoubleRow matmuls (K=256 in one pass, f32 PSUM) + one of two
    row-sum consumers, statically balanced to the engines' measured
    rates (~51us each at the 1.2 GHz uncore state):
      * 30 ACT units (W=1536/512): exp via ScalarE ACTIVATE with the
        free in-op accumulator (accum_out) - one fused pass.
      * 22 DVE units (W=1024/512): Schraudolph fast-exp on DVE
        (int-converting multiply-add to an i32 whose bits are the f32
        exp) + bitcast tensor_reduce. Placed on target slabs that
        exclude both own-image diagonal bands, so the -inf correction
        on the host subtracts exact exps.
    Units are ordered target-range-major so the first 8 units of each
    engine reuse one input chunk. PSUM: ACT ping-pongs over its own
    2x[128,1536] slots; DVE needs only ONE [128,1024] slot (its refill
    hides under the previous unit's tensor_reduce, which reads SBUF).
    6+2 = 8 banks, and no slot is ever handed between engines.
  - Host: the 16x16 own-image diagonal dot blocks (recomputed from the
    same fp8 inputs, ~0.4% of total FLOPs), masks from the roi indices,
    positive-pair sums, the -inf masking correction (subtract the exp of
    masked entries from the denominators), log, and the final mean.
"""
import numpy as np
import ml_dtypes

import concourse.bacc as bacc
import concourse.mybir as mybir
import concourse.tile as tile
from concourse.bass_utils import run_bass_kernel_spmd

TEMP = 0.1
EPS = 1e-11
SCALE = float(np.float32(1.0 / (TEMP + EPS)))
NCORES = 8
B, N, D = 256, 16, 256
R = B * N          # 4096 flat rows
RPC = R // NCORES  # 512 rows per core
BF16 = mybir.dt.bfloat16
FP8 = mybir.dt.float8e4
NPFP8 = ml_dtypes.float8_e4m3
F32 = mybir.dt.float32
I32 = mybir.dt.int32
# Schraudolph fast-exp: exp(s*x) ~= bitcast_f32(int32(x*SA + SB))
SA = float(np.float32((2**23 / np.log(2.0)) * (1.0 / (0.1 + 1e-11))))
SB = float(np.float32(127 * 2**23 - 486411))

# Per (pt, tsel) side (4096 target cols), one of three layouts.  The all-D
# and hybrid sides must keep the own-image diagonal (aa: t1 cols [0,512)
# for px=0, bb: t2 for px=1) on an A slab - hybrids do (diag is in A:0-1536).
ALL_D_SIDES = {(3, 1), (7, 0)}
HYBRID_SIDES = {(1, 0)}


def side_units(pt, ts):
    if (pt, ts) in ALL_D_SIDES:
        return [(pt, ts, c, 1024, "D") for c in (0, 1024, 2048, 3072)]
    if (pt, ts) in HYBRID_SIDES:
        return [(pt, ts, 0, 1536, "A"), (pt, ts, 1536, 1536, "A"),
                (pt, ts, 3072, 512, "A"), (pt, ts, 3584, 512, "D")]
    return [(pt, ts, 0, 1536, "A"), (pt, ts, 1536, 1536, "A"),
            (pt, ts, 3072, 1024, "D")]


def ucost(w, kind):
    return (w + 352) / 1.2 + 190 if kind == "A" else 2.25 * w + 116


def unit_sequence():
    """Target-range-major queues per engine, merged by cumulative engine
    time so both engines are fed from the start and finish together."""
    units = [u for pt in range(8) for ts in range(2)
             for u in side_units(pt, ts)]
    a = sorted([u for u in units if u[4] == "A"],
               key=lambda u: (u[1], u[2], u[0]))
    d = sorted([u for u in units if u[4] == "D"],
               key=lambda u: (u[1], u[2], u[0]))
    seq, ta, td = [], 0.0, 0.0
    while a or d:
        ca = ta + ucost(a[0][3], "A") if a else None
        cd = td + ucost(d[0][3], "D") if d else None
        if a and (not d or ca <= cd):
            seq.append(a.pop(0))
            ta = ca
        else:
            seq.append(d.pop(0))
            td = cd
    # D-first head: the first D unit's 2 matmuls run before the first A
    # unit's 3 on the cold PE, starting the (otherwise last-closing) DVE
    # ~2us earlier for ~0.5us of ACT start.
    if seq[0][4] == "A" and seq[1][4] == "D":
        seq[0], seq[1] = seq[1], seq[0]
    return seq


def build_layout():
    """Pack order of the single input tensor: pred tiles (256B) at first
    use, 512-col target chunk-pairs (k0|k1, 1024B) at first use."""
    seq = unit_sequence()
    p_off, t_off, atoms = {}, {}, []
    off = 0
    for pt, ts, c0, w, _ in seq:
        if pt not in p_off:
            p_off[pt] = off
            atoms.append((off, 256))
            off += 256
        for c in range(c0, c0 + w, 512):
            if (ts, c) not in t_off:
                t_off[(ts, c)] = off
                atoms.append((off, 1024))
                off += 1024
    return seq, p_off, t_off, atoms, off


SEQ, P_OFF, T_OFF, ATOMS, INP_BYTES = build_layout()
UCOL = {u[:3]: i for i, u in enumerate(SEQ)}


def dma_segments():
    """Split the packed stream into segments cut exactly at the first two
    units' data boundaries (seg0 = all of unit 0's data, first on the SP
    queue; seg1 = unit 1's, first on the ACT queue - so the scheduler's
    DMA-readiness model orders the first consumers first), then
    ~4KB-per-partition runs striped over both queues."""
    early = [2304, 3584]
    cuts, acc = [], 0
    for off, size in ATOMS:
        acc += size
        if len(cuts) < len(early) and acc >= early[len(cuts)]:
            cuts.append(off + size)
        elif len(cuts) >= len(early) and acc - cuts[-1] >= 4096:
            cuts.append(off + size)
    if not cuts or cuts[-1] != INP_BYTES:
        cuts.append(INP_BYTES)
    segs, lo = [], 0
    for hi in cuts:
        segs.append((lo, hi))
        lo = hi
    return segs


def build_nc():
    """Build + schedule + compile the SPMD per-core Bass program."""
    nc = bacc.Bacc("TRN2", target_bir_lowering=False, debug=False,
                   num_devices=NCORES)

    inp_dram = nc.dram_tensor("inp", [128, INP_BYTES], FP8,
                              kind="ExternalInput")
    sacc = nc.dram_tensor("sacc", [128, 64], F32, kind="ExternalOutput")

    with tile.TileContext(nc) as tc:
        with (
            tc.tile_pool(name="const", bufs=1) as const_pool,
            tc.tile_pool(name="psum", bufs=1, space="PSUM") as psum_pool,
        ):
            inp_sb = const_pool.tile([128, INP_BYTES], FP8, name="inp_sb",
                                     tag="inp")
            strip = const_pool.tile([128, 64], F32, name="strip", tag="strip")
            zbias = const_pool.tile([128, 1], F32, name="zbias", tag="zbias")
            scr = const_pool.tile([128, 1536], BF16, name="scr", tag="scr")
            # manual 2-buffer rotation (a dedicated pool would add another
            # pool-teardown barrier to the exit path)
            schs = [const_pool.tile([128, 1024], I32, name=f"sch{i}",
                                    tag=f"sch{i}") for i in range(2)]
            # Input DMA segments first in program order, striped over the two
            # HWDGE queues (the early ACT-queue dma_start issues run before
            # the warm-up ACTIVATE and overlap the data wait).
            for i, (lo, hi) in enumerate(dma_segments()):
                eng = nc.scalar if i in (1, 3, 5) else nc.sync
                eng.dma_start(out=inp_sb[:, lo:hi], in_=inp_dram[:, lo:hi])

            # Explicit zero-bias AP, zeroed ON ScalarE so the dependency is
            # engine-internal program order (a DVE memset would chain the
            # first ACTIVATE behind a coarsened cross-engine semaphore that
            # only clears at the DVE's second Schraudolph op, ~3.5us late).
            # The strip needs NO init at all: accum_out and tensor_reduce
            # overwrite their outputs.
            nc.scalar.memzero(zbias)

            # PSUM: ACT pair 2x[128,1536] (6 banks) + one DVE slot (2 banks).
            psA = [psum_pool.tile([128, 1536], F32, name=f"psA{i}", tag=f"psA{i}")
                   for i in range(2)]
            psD = psum_pool.tile([128, 1024], F32, name="psD", tag="psD")

            # Warm the exp table set during the input-DMA window so the first
            # real ACTIVATE does not pay the ~2.7us ACT_TABLE_LOAD (input and
            # output are scratch - exp of garbage, never read).
            warm = const_pool.tile([128, 2], F32, name="warm", tag="warm")
            nc.scalar.memzero(warm)
            nc.scalar.activation(warm, warm,
                                 mybir.ActivationFunctionType.Exp, bias=zbias)
            # Zero the strip's never-written spare columns on ScalarE too
            # (same-engine ordering, no semaphore on the first EXP's path).
            nc.scalar.memzero(strip)

            def lhs_ap(pt):
                o = P_OFF[pt]
                return inp_sb[:, o:o + 256].rearrange("p (k c) -> p k c", k=2)

            def rhs_ap(ts, c):
                # 512-col moving chunk-pair (k0|k1) at (ts, c)
                o = T_OFF[(ts, c)]
                return inp_sb[:, o:o + 1024].rearrange("p (k c) -> p k c", k=2)

            na = nd = 0
            for i, (pt, ts, c0, w, kind) in enumerate(SEQ):
                ps = psA[na % 2] if kind == "A" else psD
                lhs = lhs_ap(pt)
                # fp8 DoubleRow: both 128-deep K chunks contract in a single
                # pass (lhsT/rhs carry the k pair on a middle AP dim).
                for j in range(0, w, 512):
                    nc.tensor.matmul(
                        ps[:, j:j + 512],
                        lhs, rhs_ap(ts, c0 + j),
                        start=True, stop=True,
                        perf_mode=mybir.MatmulPerfMode.DoubleRow)
                if kind == "A":
                    nc.scalar.activation(
                        scr[:, 0:w], ps[:, 0:w],
                        mybir.ActivationFunctionType.Exp,
                        bias=zbias, scale=SCALE,
                        accum_out=strip[:, i:i + 1])
                    na += 1
                else:
                    sch = schs[nd % 2]
                    nd += 1
                    nc.vector.tensor_scalar(
                        sch[:, 0:w], ps[:, 0:w], SA, SB,
                        op0=mybir.AluOpType.mult, op1=mybir.AluOpType.add)
                    nc.vector.tensor_reduce(
                        strip[:, i:i + 1], sch.bitcast(F32)[:, 0:w],
                        axis=mybir.AxisListType.X, op=mybir.AluOpType.add)
            # Final strip DMA on the sync HWDGE queue: drains in ~0.1us at
            # kernel exit (the gpsimd SWDGE path would take ~2.4us).
            nc.sync.dma_start(out=sacc.ap(), in_=strip)

    nc.compile()
    return nc


_NC = None


def _get_nc():
    global _NC
    if _NC is None:
        _NC = build_nc()
    return _NC


def _l2norm(x):
    return x / np.linalg.norm(x, axis=-1, keepdims=True)


def prep_arrays(pred1, pred2, target1, target2):
    """fp8 transposed operands, shared by host_prep and the sim check."""
    p1t = _l2norm(np.asarray(pred1, np.float32)).reshape(R, D).T.astype(NPFP8)
    p2t = _l2norm(np.asarray(pred2, np.float32)).reshape(R, D).T.astype(NPFP8)
    t1t = _l2norm(np.asarray(target1, np.float32)).reshape(R, D).T.astype(NPFP8)
    t2t = _l2norm(np.asarray(target2, np.float32)).reshape(R, D).T.astype(NPFP8)
    return p1t, p2t, t1t, t2t


def pack_core(p1t, p2t, t1t, t2t, c):
    """Pack core c's inputs into the need-ordered [128, INP_BYTES] tensor."""
    r0 = c * RPC
    t = [np.concatenate([t1t[:, r0:], t1t[:, :r0]], axis=1),
         np.concatenate([t2t[:, r0:], t2t[:, :r0]], axis=1)]
    pcat = np.concatenate([p1t[:, r0:r0 + RPC], p2t[:, r0:r0 + RPC]], axis=1)
    inp = np.zeros((128, INP_BYTES), NPFP8)
    for pt, o in P_OFF.items():
        for k in range(2):
            inp[:, o + k * 128:o + (k + 1) * 128] = \
                pcat[k * 128:(k + 1) * 128, pt * 128:(pt + 1) * 128]
    for (ts, cc), o in T_OFF.items():
        for k in range(2):
            inp[:, o + k * 512:o + (k + 1) * 512] = \
                t[ts][k * 128:(k + 1) * 128, cc:cc + 512]
    return inp


def host_prep(pred1, pred2, target1, target2):
    p1t, p2t, t1t, t2t = prep_arrays(pred1, pred2, target1, target2)
    # Raw own-image diagonal dot blocks (b, n, m), fp8-quantized operands in
    # f32 - the same products the device computes, ~0.4% of total FLOPs.
    pf = [p1t.T.astype(np.float32).reshape(B, N, D),
          p2t.T.astype(np.float32).reshape(B, N, D)]
    tf = [t1t.T.astype(np.float32).reshape(B, N, D),
          t2t.T.astype(np.float32).reshape(B, N, D)]
    diag = [[np.einsum('bnd,bmd->bnm', pf[px], tf[ts]).astype(np.float32)
             for ts in range(2)] for px in range(2)]
    in_maps = [{"inp": pack_core(p1t, p2t, t1t, t2t, c)}
               for c in range(NCORES)]
    return in_maps, diag


def host_post(results, diag, pind1, pind2, tind1, tind2):
    # S[px, pred]: denominator sums of exp over all 8192 targets.
    S = np.zeros((2, R), np.float64)
    cols = {pt: [c for (p_, t_, c0), c in UCOL.items() if p_ == pt]
            for pt in range(8)}
    for c, res in enumerate(results):
        r0 = c * RPC
        sa = np.asarray(res["sacc"]).astype(np.float64)   # [128, 64]
        for pt in range(8):
            px, mt = pt // 4, pt % 4
            rows = r0 + mt * 128
            S[px, rows:rows + 128] += sa[:, cols[pt]].sum(axis=1)

    sc = np.float32(SCALE)
    D_aa = sc * diag[0][0]
    D_ab = sc * diag[0][1]
    D_ba = sc * diag[1][0]
    D_bb = sc * diag[1][1]

    f32 = np.float32
    pind1, pind2 = np.asarray(pind1), np.asarray(pind2)
    tind1, tind2 = np.asarray(tind1), np.asarray(tind2)
    same_aa = (pind1[:, :, None] == tind1[:, None, :]).astype(f32)
    same_ab = (pind1[:, :, None] == tind2[:, None, :]).astype(f32)
    same_ba = (pind2[:, :, None] == tind1[:, None, :]).astype(f32)
    same_bb = (pind2[:, :, None] == tind2[:, None, :]).astype(f32)

    S0 = S[0].reshape(B, N)
    S1 = S[1].reshape(B, N)
    # -inf masking correction: both diagonal bands live on ACT units, so
    # the device added exact exps - subtract exact exps.
    corr0 = (same_aa * np.exp(D_aa.astype(np.float64))).sum(-1)
    corr1 = (same_bb * np.exp(D_bb.astype(np.float64))).sum(-1)
    lse0 = np.log(S0 - corr0)
    lse1 = np.log(S1 - corr1)

    num_pos0 = same_ab.sum(-1)
    num_pos1 = same_ba.sum(-1)
    pos_sum0 = (same_ab * D_ab).sum(-1)
    pos_sum1 = (same_ba * D_ba).sum(-1)

    area0 = (pind1[:, :, None] == pind1[:, None, :]).astype(f32).sum(-1)
    area1 = (pind2[:, :, None] == pind2[:, None, :]).astype(f32).sum(-1)
    w0 = (num_pos0 > 0.001).astype(f32) / area0
    w1 = (num_pos1 > 0.001).astype(f32) / area1

    ce0 = -w0 * (pos_sum0 - num_pos0 * lse0) / np.maximum(num_pos0, 1.0)
    ce1 = -w1 * (pos_sum1 - num_pos1 * lse1) / np.maximum(num_pos1, 1.0)
    return np.float32(ce0.mean() + ce1.mean())


def run_hw(inputs, trace=False):
    nc = _get_nc()
    in_maps, diag = host_prep(inputs["pred1"], inputs["pred2"],
                              inputs["target1"], inputs["target2"])
    last_err = None
    for attempt in range(3):
        try:
            res = run_bass_kernel_spmd(nc, in_maps,
                                       core_ids=list(range(NCORES)),
                                       trace=trace)
            break
        except Exception as e:  # transient NRT device errors recover on retry
            last_err = e
            import time
            time.sleep(20 * (attempt + 1))
    else:
        raise last_err
    loss = host_post(res.results, diag, inputs["pind1"],
                     inputs["pind2"], inputs["tind1"], inputs["tind2"])
    return loss, res


def kernel(**inputs):
    loss, _ = run_hw(inputs, trace=False)
    return loss
